# revision 3
# baseline (speedup 1.0000x reference)
"""Trainium2 Bass kernel for the CellularAutomata step (dense_cnn).

Math (per pixel): s = depthwise3x3(wrap_pad(x), [identity, sobel_x, sobel_y]);
h = relu(s @ W1 + b1); out = clip(x + h @ W2 + b2, 0, 1).

Strategy (pure data parallel, batch -> 8 cores, weights replicated):
  - Host: per-core image to channel-major flat layout [16, 258*258] with wrap
    padding; the whole output (with junk wrap columns) is computed on a padded
    flat grid and the host slices out the valid 256x256 region.
  - The 3x3 perception conv + W1 are folded (host-side) into three [48, 128]
    matrices, one per vertical tap dy.  The device loads x three times at flat
    offsets +0/+1/+2 onto partition blocks 0-15/16-31/32-47, so each dy is a
    single K=48 matmul whose free-dim offset dy*258 walks the rows; the three
    dy matmuls accumulate in PSUM.  float32r -> full PE rate at N=512.
  - The residual "+x" is an extra K=16 identity matmul into the dx PSUM.
    dx for 4 consecutive 512-pixel chunks is stacked at PSUM partition strips
    0/32/64/96 (explicit tile_position), so bias+clip post-ops run on 128
    partitions on the DVE, then one DMA writes all 4 chunks out.
"""

import numpy as np
from contextlib import ExitStack

import concourse.bass as bass
import concourse.tile as tile
from concourse import bacc, mybir
from concourse.bass_utils import run_bass_kernel_spmd

B, S, C, HID = 8, 256, 16, 128
N_CORES = 8
P = S + 2                    # padded width = 258
FLAT = P * P                 # 66564
CH = 512                     # pixels per chunk
BF = 4                       # chunks per block (shared DMA)
NCHUNK = 130                 # covers all valid padded-flat positions
NB = (NCHUNK + BF - 1) // BF
SPAN = (BF - 1) * CH + CH + 2 * P + 8   # block free extent read by matmuls
XLEN = 144 * CH + 2 * P + 16            # padded flat x length (covers bf<=16)

_CACHE = {}


def _build_program(bf=BF, xx_bufs=6, h_bufs=4, u_bufs=3, o_bufs=4,
                   ph_bufs=2, pdx_bufs=4, repeat=1, use_gpsimd=False, mode='full',
                   use_bf16=False, act_every=1000, relu_mode='act', probe=None):
    f32 = mybir.dt.float32
    f32r = mybir.dt.float32r
    Relu = mybir.ActivationFunctionType.Relu
    add = mybir.AluOpType.add
    op_max = mybir.AluOpType.max
    op_min = mybir.AluOpType.min

    nc = bacc.Bacc("TRN2", target_bir_lowering=False, debug=False,
                   num_devices=N_CORES)

    bf16 = mybir.dt.bfloat16
    mdt = bf16 if use_bf16 else f32r
    xf = nc.dram_tensor("xf", [C, XLEN], f32r, kind="ExternalInput").ap()
    xfb = (nc.dram_tensor("xfb", [C, XLEN], bf16, kind="ExternalInput").ap()
           if use_bf16 else None)
    wc = nc.dram_tensor("wc", [48, 3 * HID], mdt, kind="ExternalInput").ap()
    w2 = nc.dram_tensor("w2", [HID, 32], mdt, kind="ExternalInput").ap()
    b1 = nc.dram_tensor("b1", [HID, 1], f32, kind="ExternalInput").ap()
    b2s = nc.dram_tensor("b2s", [C, 1], f32, kind="ExternalInput").ap()
    out = nc.dram_tensor("out", [C, XLEN], f32, kind="ExternalOutput").ap()

    with tile.TileContext(nc) as tc, ExitStack() as ctx:
        wpool = ctx.enter_context(tc.tile_pool(name="wts", bufs=1))
        wc_sb = wpool.tile([48, 3 * HID], mdt)
        nc.sync.dma_start(wc_sb[:], wc)
        w2_sb = wpool.tile([HID, 32], mdt)
        nc.sync.dma_start(w2_sb[:], w2)
        b1_sb = wpool.tile([HID, 1], f32)
        nc.sync.dma_start(b1_sb[:], b1)
        b2_sb = wpool.tile([C, 1], f32)
        nc.sync.dma_start(b2_sb[:], b2s)

        xpool = ctx.enter_context(tc.tile_pool(name="xx", bufs=xx_bufs))
        xcpool = ctx.enter_context(tc.tile_pool(name="xc", bufs=xx_bufs))
        hpool = ctx.enter_context(tc.tile_pool(name="h", bufs=h_bufs))
        upool = ctx.enter_context(tc.tile_pool(name="u", bufs=u_bufs))
        opool = ctx.enter_context(tc.tile_pool(name="o", bufs=o_bufs))
        ph_pool = ctx.enter_context(tc.tile_pool(name="ph", bufs=ph_bufs, space="PSUM"))
        pdx_pool = ctx.enter_context(tc.tile_pool(name="pdx", bufs=pdx_bufs, space="PSUM"))

        nblocks = (NCHUNK + bf - 1) // bf
        span = (bf - 1) * CH + CH + 2 * P + 8
        rep_cm = tc.For_i(0, repeat, 1) if repeat > 1 else None
        if rep_cm is not None:
            rep_cm.__enter__()

        def load_xx(b):
            # x triple in ONE DMA: overlapping-window source AP
            # dst partition 16*dc+cc <- xsrc[cc, b*bf*CH + dc + f]
            q0 = b * bf * CH
            xsrc = xfb if use_bf16 else xf
            xx = xpool.tile([48, span], mdt, tag="xx")
            base = xsrc[:, q0:q0 + span]
            src = bass.AP(tensor=base.tensor, offset=base.offset,
                          ap=[[1, 3]] + [list(p) for p in base.ap])
            nc.sync.dma_start(xx[:], src)
            xc = None
            if use_bf16:
                # exact f32 center strip for the residual add
                xc = xcpool.tile([C, bf * CH], f32r, tag="xc")
                nc.sync.dma_start(xc[:], xf[:, q0 + P + 1:q0 + P + 1 + bf * CH])
            return xx, xc

        xx, xc = load_xx(0)
        pending_out = None   # (o_tile, p0) emitted one block later
        for b in range(nblocks):
            p0 = b * bf * CH
            xx_next, xc_next = load_xx(b + 1) if b + 1 < nblocks else (None, None)
            if pending_out is not None:
                po, pp0 = pending_out
                nc.sync.dma_start(
                    out[:, pp0 + P + 1:pp0 + P + 1 + bf * CH], po[:])

            o = opool.tile([C, bf * CH], f32)
            hs = []
            # phase 1: all tap matmuls (PE) + relu (ACT) — keeps the PE
            # queue free of instructions that wait on other engines.
            # chunks are processed in pairs sharing a 2-bank PSUM tile so
            # one ACT relu covers 1024 columns.
            for sp in range(bf // 2):
                ph = ph_pool.tile([HID, 2 * CH], f32)
                ntap = 1 if probe == 'taps1' else 3
                for s2 in range(2):
                    f0 = (2 * sp + s2) * CH
                    for dy in range(ntap):
                        nc.tensor.matmul(
                            ph[:, s2 * CH:(s2 + 1) * CH],
                            lhsT=wc_sb[:, dy * HID:(dy + 1) * HID],
                            rhs=xx[:, f0 + dy * P:f0 + dy * P + CH],
                            start=(dy == 0), stop=(dy == ntap - 1),
                        )
                h = hpool.tile([HID, 2 * CH], mdt)
                nc.scalar.activation(h[:], ph[:], Relu, bias=b1_sb[:])
                hs.append(h)
            # phase 2: MLP2 matmuls (PE), then post-ops (DVE)
            pdxs = []
            for s in range(bf):
                pdx = pdx_pool.tile([32, CH], f32)
                nc.tensor.matmul(pdx[:], lhsT=w2_sb[:],
                                 rhs=hs[s // 2][:, (s % 2) * CH:(s % 2 + 1) * CH],
                                 start=True, stop=True)
                pdxs.append(pdx)
            if mode == 'full':
                # u = dx + x_center per chunk, then block-wide
                # o = min(max(u + b2, 0), 1) in two fused DVE ops
                u = upool.tile([C, bf * CH], f32)
                for s in range(bf):
                    f0 = s * CH
                    if use_bf16:
                        res_src = xc[:, f0:f0 + CH].bitcast(f32)
                    else:
                        res_src = xx[0:C, f0 + P + 1:f0 + P + 1 + CH].bitcast(f32)
                    if probe == 'nott':
                        nc.vector.tensor_copy(u[:, f0:f0 + CH], pdxs[s][0:C, :])
                    else:
                        nc.vector.tensor_tensor(
                            u[:, f0:f0 + CH], pdxs[s][0:C, :], res_src, op=add)
                nc.vector.tensor_scalar(u[:], u[:], b2_sb[:], 0.0,
                                        op0=add, op1=op_max)
                nc.vector.tensor_scalar_min(o[:], u[:], 1.0)

            pending_out = (o, p0)
            xx, xc = xx_next, xc_next
        po, pp0 = pending_out
        nc.sync.dma_start(out[:, pp0 + P + 1:pp0 + P + 1 + bf * CH], po[:])
        if rep_cm is not None:
            rep_cm.__exit__(None, None, None)

    nc.compile()
    return nc


def _prep_weights(pk, W1):
    # pk [3(dy),3(dx),3(k)]; W1 [48,128] rows indexed 3*ci+k
    W1r = W1.reshape(C, 3, HID)                      # [ci, k, hid]
    Wfull = np.einsum("ydk,ckh->ydch", pk, W1r)      # [dy, dx, ci, hid]
    return np.ascontiguousarray(
        np.concatenate([Wfull[0].reshape(3 * C, HID),
                        Wfull[1].reshape(3 * C, HID),
                        Wfull[2].reshape(3 * C, HID)], axis=1), dtype=np.float32)


def kernel(x, perception_kernel, W1, b1, W2, b2):
    x = np.asarray(x, dtype=np.float32)
    pk = np.asarray(perception_kernel, dtype=np.float32)
    W1 = np.asarray(W1, dtype=np.float32)
    b1 = np.asarray(b1, dtype=np.float32)
    W2 = np.asarray(W2, dtype=np.float32)
    b2 = np.asarray(b2, dtype=np.float32)

    if "nc" not in _CACHE:
        _CACHE["nc"] = _build_program()
    nc = _CACHE["nc"]

    wc_np = _prep_weights(pk, W1)
    w2_np = np.zeros((HID, 32), np.float32)
    w2_np[:, :C] = W2
    b1_np = np.ascontiguousarray(b1.reshape(HID, 1))
    b2s = np.ascontiguousarray(b2.reshape(C, 1))

    in_maps = []
    for c in range(N_CORES):
        xt = np.ascontiguousarray(x[c].transpose(2, 0, 1))      # [C, S, S]
        xt = np.pad(xt, ((0, 0), (1, 1), (1, 1)), mode="wrap")  # [C, 258, 258]
        xflat = np.zeros((C, XLEN), np.float32)
        xflat[:, :FLAT] = xt.reshape(C, FLAT)
        in_maps.append({
            "xf": xflat, "wc": wc_np, "w2": w2_np,
            "b1": b1_np, "b2s": b2s,
        })

    import os as _os
    _trace = bool(int(_os.environ.get("KTRACE", "0")))
    if _trace:
        import tempfile as _tempfile
        from trn_agent_boot.trn_boot import _ntff_profile_via_ctypes
        _hook = _ntff_profile_via_ctypes('/opt/axon/libaxon_pjrt.so')
        _neff_dir = _tempfile.mkdtemp(prefix="ktrace_")
        with _hook(_neff_dir, [0]):
            res = run_bass_kernel_spmd(nc, in_maps, list(range(N_CORES)))
        _CACHE["neff_dir"] = _neff_dir
        _CACHE["nc_obj"] = nc
    else:
        res = run_bass_kernel_spmd(nc, in_maps, list(range(N_CORES)))
    _CACHE["exec_time_ns"] = getattr(res, "exec_time_ns", None)
    _CACHE["trace"] = getattr(res, "instructions_and_trace", None)
    outs = []
    for c in range(N_CORES):
        of = res.results[c]["out"][:, :FLAT].reshape(C, P, P)
        outs.append(of[:, 1:S + 1, 1:S + 1].transpose(1, 2, 0))
    return np.ascontiguousarray(np.stack(outs, axis=0), dtype=np.float32)

